# revision 5
# baseline (speedup 1.0000x reference)
"""TRN2 Bass kernel for nn_ClassSemantic (scatter_memory).

Strategy
--------
Data-parallel over batch: core k owns samples 4k..4k+3 and runs
projection (1x1 conv) + memory-gather attention + concat, all in fp32r
(TF32-like, ~13 mantissa bits, 4x faster PE than fp32).

The sequential EMA queue update depends on the per-sample masked
feature means only, which are algebraically separable:
    feat_b = mean_hw((Wp@f + bp) * pred) = Wp @ mean_hw(f * pred) + bp * mean(pred)
The inner reduction (134 MFLOP, 0.4% of total work) is computed on the
host, then the exactly-sequential 32-step EMA scan (tiny: [4,20,256]
state) runs on the host in float64 and the final queue rows are shipped
to every core as constants.  The device never needs a collective.

Softmax over the 20 memory slots: logits are empirically in [-3, 3]
(queue rows are ~unit-norm, x ~ N(0,1)), so exp() without max
subtraction is safe.  Column sums / broadcasts across the 20-partition
axis are done with tiny ones-matmuls on the PE.
"""
import os
import numpy as np
from contextlib import ExitStack

B, IN_C, H, W_SP = 32, 512, 64, 64
CODE, CLASSES, MEM = 256, 4, 20
HW = H * W_SP              # 4096
NCORES = 8
BPC = B // NCORES          # 4 samples per core
DECAY, EPS = 0.9, 1e-12
NCH = 8                    # n-chunks per sample
NT = HW // NCH             # 512 spatial positions per chunk

_PROGRAM_CACHE = {}
LAST_RESULTS = None        # stash for test harness introspection


def _host_queue_update(feats, preds, labels, flag, queue, Wp, bp):
    """Final queue after the reference's sequential EMA scan (float64)."""
    if int(flag) != 1:
        return queue.astype(np.float32)
    f3 = feats.reshape(B, IN_C, HW)
    p2 = preds.reshape(B, HW)
    # g_b = mean_n feats_b[:, n] * pred_b[n]  (batched sgemv)
    g = np.matmul(f3, p2[:, :, None])[:, :, 0] / np.float32(HW)
    feat = g @ Wp.T.astype(np.float32) + bp[None, :] * p2.mean(axis=1)[:, None]
    q = queue.astype(np.float64)
    for i in range(B):
        l = int(labels[i])
        f = feat[i].astype(np.float64)
        slot = q[l]
        logit = slot @ f
        upd = logit[:, None] * f[None, :]
        nrm = np.sqrt((upd * upd).sum(axis=1, keepdims=True))
        upd = upd / np.maximum(nrm, EPS)
        q[l] = DECAY * slot + (1.0 - DECAY) * upd
    return q.astype(np.float32)


def _build_program():
    from concourse import bacc, mybir
    import concourse.tile as tile

    f32, f32r = mybir.dt.float32, mybir.dt.float32r
    nc = bacc.Bacc("TRN2", target_bir_lowering=False, debug=False)

    feats_in = nc.dram_tensor("feats", [BPC, IN_C, HW], f32r, kind="ExternalInput").ap()
    wpt_in = nc.dram_tensor("wpt", [IN_C, CODE], f32r, kind="ExternalInput").ap()
    bp_in = nc.dram_tensor("bpc", [128, 2], f32, kind="ExternalInput").ap()
    qat_in = nc.dram_tensor("qat", [BPC, CODE, MEM], f32r, kind="ExternalInput").ap()
    qa_in = nc.dram_tensor("qa", [BPC, MEM, CODE], f32r, kind="ExternalInput").ap()
    ones20_in = nc.dram_tensor("ones20", [MEM, 1], f32r, kind="ExternalInput").ap()
    ones1_in = nc.dram_tensor("ones1", [1, MEM], f32r, kind="ExternalInput").ap()
    out_ext = nc.dram_tensor("out", [BPC, 2 * CODE, HW], f32, kind="ExternalOutput").ap()

    with tile.TileContext(nc) as tc, ExitStack() as ctx:
        consts = ctx.enter_context(tc.tile_pool(name="consts", bufs=1))
        fpool = ctx.enter_context(tc.tile_pool(name="fpool", bufs=3))
        xpool = ctx.enter_context(tc.tile_pool(name="xpool", bufs=2))
        upool = ctx.enter_context(tc.tile_pool(name="upool", bufs=2))
        spool = ctx.enter_context(tc.tile_pool(name="spool", bufs=2))
        ppp = ctx.enter_context(tc.tile_pool(name="ppp", bufs=2, space="PSUM"))
        pps = ctx.enter_context(tc.tile_pool(name="pps", bufs=1, space="PSUM"))
        ppu = ctx.enter_context(tc.tile_pool(name="ppu", bufs=2, space="PSUM"))

        # constants
        wpt_sb = consts.tile([128, 4, CODE], f32r, name="wpt_sb")       # [p, kchunk, o]
        nc.sync.dma_start(wpt_sb[:], wpt_in.rearrange("(kk p) m -> p kk m", p=128))
        bp_sb = consts.tile([128, 2], f32, name="bp_sb")                # [p, half]
        nc.sync.dma_start(bp_sb[:], bp_in[:])
        qat_sb = consts.tile([128, BPC, 2, MEM], f32r, name="qat_sb")   # [p, b, kchunk, m]
        nc.sync.dma_start(qat_sb[:], qat_in.rearrange("b (kk p) m -> p b kk m", p=128))
        qa_sb = consts.tile([MEM, BPC, CODE], f32r, name="qa_sb")       # [m, b, c]
        nc.sync.dma_start(qa_sb[:], qa_in.rearrange("b m c -> m b c"))
        ones20_sb = consts.tile([MEM, 1], f32r, name="ones20_sb")
        nc.sync.dma_start(ones20_sb[:], ones20_in[:])
        ones1_sb = consts.tile([1, MEM], f32r, name="ones1_sb")
        nc.sync.dma_start(ones1_sb[:], ones1_in[:])

        for b in range(BPC):
            x_sb = xpool.tile([128, 2, HW], f32r, tag="x_sb")
            u_sb = upool.tile([128, 2, HW], f32, tag="u_sb")
            feats_b = feats_in[b].rearrange("(kk p) n -> p kk n", p=128)

            # ---- projection: x = WpT.T @ feats + bp ----
            for j in range(NCH):
                ft = fpool.tile([128, 4, NT], f32r, tag="ft")
                nc.sync.dma_start(ft[:], feats_b[:, :, j * NT:(j + 1) * NT])
                for h in range(2):
                    ps = ppp.tile([128, NT], f32, tag="proj_ps")
                    for kk in range(4):
                        nc.tensor.matmul(
                            ps[:], wpt_sb[:, kk, h * 128:(h + 1) * 128], ft[:, kk, :],
                            start=(kk == 0), stop=(kk == 3))
                    # psum -> sbuf with per-channel bias; fp32r out rounds for PE
                    nc.vector.tensor_scalar_add(
                        x_sb[:, h, j * NT:(j + 1) * NT], ps[:], bp_sb[:, h:h + 1])

            # ---- attention: softmax over 20 memory slots, per position ----
            for j in range(NCH):
                js = slice(j * NT, (j + 1) * NT)
                lg = pps.tile([MEM, NT], f32, tag="logit_ps")
                for kk in range(2):
                    nc.tensor.matmul(lg[:], qat_sb[:, b, kk, :], x_sb[:, kk, js],
                                     start=(kk == 0), stop=(kk == 1))
                pexp = spool.tile([MEM, NT], f32r, tag="pexp")
                nc.scalar.activation(pexp[:], lg[:], mybir.ActivationFunctionType.Exp)
                cs = pps.tile([1, NT], f32, tag="colsum_ps")
                nc.tensor.matmul(cs[:], ones20_sb[:], pexp[:], start=True, stop=True)
                rc = spool.tile([1, NT], f32r, tag="recip")
                with nc.allow_low_precision(reason="fp32r rounding for PE ingest"):
                    nc.vector.reciprocal(rc[:], cs[:])
                bc = pps.tile([MEM, NT], f32, tag="bcast_ps")
                nc.tensor.matmul(bc[:], ones1_sb[:], rc[:], start=True, stop=True)
                pn = spool.tile([MEM, NT], f32r, tag="pn")
                nc.vector.tensor_mul(pn[:], pexp[:], bc[:])
                for h in range(2):
                    us = ppu.tile([128, NT], f32, tag="u_ps")
                    nc.tensor.matmul(us[:], qa_sb[:, b, h * 128:(h + 1) * 128], pn[:],
                                     start=True, stop=True)
                    if h == 0:
                        nc.scalar.copy(u_sb[:, h, js], us[:])
                    else:
                        nc.vector.tensor_copy(u_sb[:, h, js], us[:])

            # ---- write out: channels [0:256] = new_feat, [256:512] = x ----
            for h in range(2):
                nc.gpsimd.dma_start(out_ext[b, 256 + h * 128:256 + (h + 1) * 128, :],
                                    x_sb[:, h, :].bitcast(f32))
                nc.gpsimd.dma_start(out_ext[b, h * 128:(h + 1) * 128, :], u_sb[:, h, :])

    nc.compile()
    return nc


def kernel(feats, preds, labels, flag, queue, Wp, bp):
    from concourse.bass_utils import run_bass_kernel_spmd
    global LAST_RESULTS

    feats = np.ascontiguousarray(np.asarray(feats, dtype=np.float32))
    preds = np.ascontiguousarray(np.asarray(preds, dtype=np.float32))
    labels = np.asarray(labels).astype(np.int64)
    queue = np.ascontiguousarray(np.asarray(queue, dtype=np.float32))
    Wp = np.ascontiguousarray(np.asarray(Wp, dtype=np.float32))
    bp = np.ascontiguousarray(np.asarray(bp, dtype=np.float32))
    try:
        flag_v = int(np.asarray(flag))
    except TypeError:
        flag_v = int(flag)

    qfin = _host_queue_update(feats, preds, labels, flag_v, queue, Wp, bp)
    qA = np.ascontiguousarray(qfin[labels])                      # [B, 20, 256]
    qAT = np.ascontiguousarray(qA.transpose(0, 2, 1))            # [B, 256, 20]
    wpt = np.ascontiguousarray(Wp.T)                             # [512, 256]
    bpc = np.ascontiguousarray(bp.reshape(2, 128).T)
    ones20 = np.ones((MEM, 1), dtype=np.float32)
    ones1 = np.ones((1, MEM), dtype=np.float32)

    if "prog" not in _PROGRAM_CACHE:
        _PROGRAM_CACHE["prog"] = _build_program()
    nc = _PROGRAM_CACHE["prog"]

    f4 = feats.reshape(B, IN_C, HW)
    in_maps = []
    for k in range(NCORES):
        s = slice(k * BPC, (k + 1) * BPC)
        in_maps.append({
            "feats": np.ascontiguousarray(f4[s]),
            "wpt": wpt,
            "bpc": bpc,
            "qat": np.ascontiguousarray(qAT[s]),
            "qa": np.ascontiguousarray(qA[s]),
            "ones20": ones20,
            "ones1": ones1,
        })

    trace = bool(int(os.environ.get("KERNEL_TRACE", "0")))
    res = run_bass_kernel_spmd(nc, in_maps, core_ids=list(range(NCORES)),
                               trace=trace)
    LAST_RESULTS = res
    out = np.concatenate([res.results[k]["out"] for k in range(NCORES)], axis=0)
    return out.reshape(B, 2 * CODE, H, W_SP)


if __name__ == "__main__":
    rng = np.random.default_rng(0)
    d = np.load("/tmp/inputs.npz")
    out = kernel(d["feats"], d["preds"], d["labels"], d["flag"], d["queue"], d["Wp"], d["bp"])
    exp = np.load("/tmp/expected.npy")
    err = np.abs(out - exp)
    print("absmax err:", err.max(), "scale-rel:", err.max() / np.abs(exp).max())


# revision 12
# speedup vs baseline: 1.0434x; 1.0434x over previous
"""TRN2 Bass kernel for nn_ClassSemantic (scatter_memory).

Strategy
--------
Data-parallel over batch: core k owns samples 4k..4k+3 and runs
projection (1x1 conv) + memory-gather attention + concat, all in fp32r
(TF32-like, ~13 mantissa bits, 4x faster PE than fp32).

The sequential EMA queue update depends on the per-sample masked
feature means only, which are algebraically separable:
    feat_b = mean_hw((Wp@f + bp) * pred) = Wp @ mean_hw(f * pred) + bp * mean(pred)
The inner reduction (134 MFLOP, 0.4% of total work) is computed on the
host, then the exactly-sequential 32-step EMA scan (tiny: [4,20,256]
state) runs on the host in float64 and the final queue rows are shipped
to every core as constants.  The device never needs a collective.

Softmax over the 20 memory slots: logits are empirically in [-3, 3]
(queue rows are ~unit-norm, x ~ N(0,1)), so exp() without max
subtraction is safe.  Column sums / broadcasts across the 20-partition
axis are done with tiny ones-matmuls on the PE.
"""
import os
import numpy as np
from contextlib import ExitStack

B, IN_C, H, W_SP = 32, 512, 64, 64
CODE, CLASSES, MEM = 256, 4, 20
HW = H * W_SP              # 4096
NCORES = 8
BPC = B // NCORES          # 4 samples per core
DECAY, EPS = 0.9, 1e-12
NCH = 8                    # n-chunks per sample
NT = HW // NCH             # 512 spatial positions per chunk

_PROGRAM_CACHE = {}
LAST_RESULTS = None        # stash for test harness introspection


def _host_queue_update(feats, preds, labels, flag, queue, Wp, bp):
    """Final queue after the reference's sequential EMA scan (float64)."""
    if int(flag) != 1:
        return queue.astype(np.float32)
    f3 = feats.reshape(B, IN_C, HW)
    p2 = preds.reshape(B, HW)
    # g_b = mean_n feats_b[:, n] * pred_b[n]  (batched sgemv)
    g = np.matmul(f3, p2[:, :, None])[:, :, 0] / np.float32(HW)
    feat = g @ Wp.T.astype(np.float32) + bp[None, :] * p2.mean(axis=1)[:, None]
    q = queue.astype(np.float64)
    for i in range(B):
        l = int(labels[i])
        f = feat[i].astype(np.float64)
        slot = q[l]
        logit = slot @ f
        upd = logit[:, None] * f[None, :]
        nrm = np.sqrt((upd * upd).sum(axis=1, keepdims=True))
        upd = upd / np.maximum(nrm, EPS)
        q[l] = DECAY * slot + (1.0 - DECAY) * upd
    return q.astype(np.float32)


def _build_program():
    from concourse import bacc, mybir
    import concourse.tile as tile

    f32, f32r = mybir.dt.float32, mybir.dt.float32r
    nc = bacc.Bacc("TRN2", target_bir_lowering=False, debug=False)

    feats_in = nc.dram_tensor("feats", [BPC, IN_C, HW], f32r, kind="ExternalInput").ap()
    wpt_in = nc.dram_tensor("wpt", [IN_C, CODE], f32r, kind="ExternalInput").ap()
    bp_in = nc.dram_tensor("bpc", [128, 2], f32, kind="ExternalInput").ap()
    qat_in = nc.dram_tensor("qat", [BPC, CODE, MEM], f32r, kind="ExternalInput").ap()
    qa_in = nc.dram_tensor("qa", [BPC, MEM, CODE], f32r, kind="ExternalInput").ap()
    ones20_in = nc.dram_tensor("ones20", [MEM, 1], f32r, kind="ExternalInput").ap()
    out_ext = nc.dram_tensor("out", [BPC, 2 * CODE, HW], f32, kind="ExternalOutput").ap()

    with tile.TileContext(nc) as tc, ExitStack() as ctx:
        consts = ctx.enter_context(tc.tile_pool(name="consts", bufs=1))
        fpool = ctx.enter_context(tc.tile_pool(name="fpool", bufs=3))
        xpool = ctx.enter_context(tc.tile_pool(name="xpool", bufs=2))
        upool = ctx.enter_context(tc.tile_pool(name="upool", bufs=2))
        spool = ctx.enter_context(tc.tile_pool(name="spool", bufs=2))
        ppp = ctx.enter_context(tc.tile_pool(name="ppp", bufs=2, space="PSUM"))
        pps = ctx.enter_context(tc.tile_pool(name="pps", bufs=2, space="PSUM"))
        ppu = ctx.enter_context(tc.tile_pool(name="ppu", bufs=2, space="PSUM"))

        # constants
        wpt_sb = consts.tile([128, 4, CODE], f32r, name="wpt_sb")       # [p, kchunk, o]
        nc.sync.dma_start(wpt_sb[:], wpt_in.rearrange("(kk p) m -> p kk m", p=128))
        bp_sb = consts.tile([128, 2], f32, name="bp_sb")                # [p, half]
        nc.sync.dma_start(bp_sb[:], bp_in[:])
        qat_sb = consts.tile([128, BPC, 2, MEM], f32r, name="qat_sb")   # [p, b, kchunk, m]
        nc.sync.dma_start(qat_sb[:], qat_in.rearrange("b (kk p) m -> p b kk m", p=128))
        qa_sb = consts.tile([MEM, BPC, CODE], f32r, name="qa_sb")       # [m, b, c]
        nc.sync.dma_start(qa_sb[:], qa_in.rearrange("b m c -> m b c"))
        ones20_sb = consts.tile([MEM, 1], f32r, name="ones20_sb")
        nc.sync.dma_start(ones20_sb[:], ones20_in[:])

        for b in range(BPC):
            x_sb = xpool.tile([128, 2, HW], f32r, tag="x_sb")
            u_sb = upool.tile([128, 2, HW], f32, tag="u_sb")
            feats_b = feats_in[b].rearrange("(kk p) n -> p kk n", p=128)

            # ---- projection: x = WpT.T @ feats + bp ----
            for j in range(NCH):
                ft = fpool.tile([128, 4, NT], f32r, tag="ft")
                nc.sync.dma_start(ft[:], feats_b[:, :, j * NT:(j + 1) * NT])
                for h in range(2):
                    ps = ppp.tile([128, NT], f32, tag="proj_ps")
                    for kk in range(4):
                        nc.tensor.matmul(
                            ps[:], wpt_sb[:, kk, h * 128:(h + 1) * 128], ft[:, kk, :],
                            start=(kk == 0), stop=(kk == 3))
                    # psum -> sbuf with per-channel bias; fp32r out rounds for PE
                    if h == 0:
                        nc.scalar.activation(
                            x_sb[:, h, j * NT:(j + 1) * NT], ps[:],
                            mybir.ActivationFunctionType.Identity,
                            bias=bp_sb[:, h:h + 1])
                    else:
                        nc.vector.tensor_scalar_add(
                            x_sb[:, h, j * NT:(j + 1) * NT], ps[:], bp_sb[:, h:h + 1])

            # ---- attention: softmax over 20 memory slots, per position ----
            for j in range(NCH):
                js = slice(j * NT, (j + 1) * NT)
                lg = pps.tile([MEM, NT], f32, tag="logit_ps")
                for kk in range(2):
                    nc.tensor.matmul(lg[:], qat_sb[:, b, kk, :], x_sb[:, kk, js],
                                     start=(kk == 0), stop=(kk == 1))
                pexp = spool.tile([MEM, NT], f32r, tag="pexp")
                nc.scalar.activation(pexp[:], lg[:], mybir.ActivationFunctionType.Exp)
                cs = pps.tile([1, NT], f32, tag="colsum_ps")
                nc.tensor.matmul(cs[:], ones20_sb[:], pexp[:], start=True, stop=True)
                # 1/colsum at ~18 correct bits (more than fp32r's mantissa),
                # then broadcast across the 20 memory-slot partitions on the
                # otherwise-idle GpSimd engine.
                rc = spool.tile([1, NT], f32, tag="recip")
                nc.vector.reciprocal_approx_fast(out=rc[:], in_=cs[:])
                rcb = spool.tile([MEM, NT], f32, tag="rcb")
                nc.gpsimd.partition_broadcast(rcb[:], rc[:], channels=MEM)
                pn = spool.tile([MEM, NT], f32r, tag="pn")
                nc.vector.tensor_mul(pn[:], pexp[:], rcb[:])
                for h in range(2):
                    us = ppu.tile([128, NT], f32, tag="u_ps")
                    nc.tensor.matmul(us[:], qa_sb[:, b, h * 128:(h + 1) * 128], pn[:],
                                     start=True, stop=True)
                    if h == 0:
                        nc.scalar.copy(u_sb[:, h, js], us[:])
                    else:
                        nc.vector.tensor_copy(u_sb[:, h, js], us[:])

            # ---- write out: channels [0:256] = new_feat, [256:512] = x ----
            for h in range(2):
                nc.gpsimd.dma_start(out_ext[b, 256 + h * 128:256 + (h + 1) * 128, :],
                                    x_sb[:, h, :].bitcast(f32))
                nc.gpsimd.dma_start(out_ext[b, h * 128:(h + 1) * 128, :], u_sb[:, h, :])

    nc.compile()
    return nc


def kernel(feats, preds, labels, flag, queue, Wp, bp):
    from concourse.bass_utils import run_bass_kernel_spmd
    global LAST_RESULTS

    feats = np.ascontiguousarray(np.asarray(feats, dtype=np.float32))
    preds = np.ascontiguousarray(np.asarray(preds, dtype=np.float32))
    labels = np.asarray(labels).astype(np.int64)
    queue = np.ascontiguousarray(np.asarray(queue, dtype=np.float32))
    Wp = np.ascontiguousarray(np.asarray(Wp, dtype=np.float32))
    bp = np.ascontiguousarray(np.asarray(bp, dtype=np.float32))
    try:
        flag_v = int(np.asarray(flag))
    except TypeError:
        flag_v = int(flag)

    qfin = _host_queue_update(feats, preds, labels, flag_v, queue, Wp, bp)
    qA = np.ascontiguousarray(qfin[labels])                      # [B, 20, 256]
    qAT = np.ascontiguousarray(qA.transpose(0, 2, 1))            # [B, 256, 20]
    wpt = np.ascontiguousarray(Wp.T)                             # [512, 256]
    bpc = np.ascontiguousarray(bp.reshape(2, 128).T)
    ones20 = np.ones((MEM, 1), dtype=np.float32)

    if "prog" not in _PROGRAM_CACHE:
        _PROGRAM_CACHE["prog"] = _build_program()
    nc = _PROGRAM_CACHE["prog"]

    f4 = feats.reshape(B, IN_C, HW)
    in_maps = []
    for k in range(NCORES):
        s = slice(k * BPC, (k + 1) * BPC)
        in_maps.append({
            "feats": np.ascontiguousarray(f4[s]),
            "wpt": wpt,
            "bpc": bpc,
            "qat": np.ascontiguousarray(qAT[s]),
            "qa": np.ascontiguousarray(qA[s]),
            "ones20": ones20,
        })

    trace = bool(int(os.environ.get("KERNEL_TRACE", "0")))
    res = run_bass_kernel_spmd(nc, in_maps, core_ids=list(range(NCORES)),
                               trace=trace)
    LAST_RESULTS = res
    out = np.concatenate([res.results[k]["out"] for k in range(NCORES)], axis=0)
    return out.reshape(B, 2 * CODE, H, W_SP)


if __name__ == "__main__":
    rng = np.random.default_rng(0)
    d = np.load("/tmp/inputs.npz")
    out = kernel(d["feats"], d["preds"], d["labels"], d["flag"], d["queue"], d["Wp"], d["bp"])
    exp = np.load("/tmp/expected.npy")
    err = np.abs(out - exp)
    print("absmax err:", err.max(), "scale-rel:", err.max() / np.abs(exp).max())


# revision 18
# speedup vs baseline: 1.0957x; 1.0501x over previous
"""TRN2 Bass kernel for nn_ClassSemantic (scatter_memory).

Strategy
--------
Data-parallel over batch: core k owns samples 4k..4k+3 and runs
projection (1x1 conv) + memory-gather attention + concat, all in fp32r
(TF32-like, ~13 mantissa bits, 4x faster PE than fp32).

The sequential EMA queue update depends on the per-sample masked
feature means only, which are algebraically separable:
    feat_b = mean_hw((Wp@f + bp) * pred) = Wp @ mean_hw(f * pred) + bp * mean(pred)
The inner reduction (134 MFLOP, 0.4% of total work) is computed on the
host, then the exactly-sequential 32-step EMA scan (tiny: [4,20,256]
state) runs on the host in float64 and the final queue rows are shipped
to every core as constants.  The device never needs a collective.

Softmax over the 20 memory slots: logits are empirically in [-3, 3]
(queue rows are ~unit-norm, x ~ N(0,1)), so exp() without max
subtraction is safe.  Column sums / broadcasts across the 20-partition
axis are done with tiny ones-matmuls on the PE.
"""
import os
import numpy as np
from contextlib import ExitStack

B, IN_C, H, W_SP = 32, 512, 64, 64
CODE, CLASSES, MEM = 256, 4, 20
HW = H * W_SP              # 4096
NCORES = 8
BPC = B // NCORES          # 4 samples per core
DECAY, EPS = 0.9, 1e-12
NCH = 8                    # n-chunks per sample
NT = HW // NCH             # 512 spatial positions per chunk

_PROGRAM_CACHE = {}
LAST_RESULTS = None        # stash for test harness introspection


def _host_queue_update(feats, preds, labels, flag, queue, Wp, bp):
    """Final queue after the reference's sequential EMA scan (float64)."""
    if int(flag) != 1:
        return queue.astype(np.float32)
    f3 = feats.reshape(B, IN_C, HW)
    p2 = preds.reshape(B, HW)
    # g_b = mean_n feats_b[:, n] * pred_b[n]  (batched sgemv)
    g = np.matmul(f3, p2[:, :, None])[:, :, 0] / np.float32(HW)
    feat = g @ Wp.T.astype(np.float32) + bp[None, :] * p2.mean(axis=1)[:, None]
    q = queue.astype(np.float64)
    for i in range(B):
        l = int(labels[i])
        f = feat[i].astype(np.float64)
        slot = q[l]
        logit = slot @ f
        upd = logit[:, None] * f[None, :]
        nrm = np.sqrt((upd * upd).sum(axis=1, keepdims=True))
        upd = upd / np.maximum(nrm, EPS)
        q[l] = DECAY * slot + (1.0 - DECAY) * upd
    return q.astype(np.float32)


def _build_program():
    from concourse import bacc, mybir
    import concourse.tile as tile

    f32, f32r = mybir.dt.float32, mybir.dt.float32r
    nc = bacc.Bacc("TRN2", target_bir_lowering=False, debug=False)

    feats_in = nc.dram_tensor("feats", [BPC, IN_C, HW], f32r, kind="ExternalInput").ap()
    wpt_in = nc.dram_tensor("wpt", [IN_C, CODE], f32r, kind="ExternalInput").ap()
    bp_in = nc.dram_tensor("bpc", [128, 2], f32, kind="ExternalInput").ap()
    qat_in = nc.dram_tensor("qat", [BPC, CODE, MEM], f32r, kind="ExternalInput").ap()
    qa_in = nc.dram_tensor("qa", [BPC, MEM, CODE], f32r, kind="ExternalInput").ap()
    ones20_in = nc.dram_tensor("ones20", [MEM, 1], f32r, kind="ExternalInput").ap()
    out_ext = nc.dram_tensor("out", [BPC, 2 * CODE, HW], f32, kind="ExternalOutput").ap()

    with tile.TileContext(nc) as tc, ExitStack() as ctx:
        consts = ctx.enter_context(tc.tile_pool(name="consts", bufs=1))
        fpool = ctx.enter_context(tc.tile_pool(name="fpool", bufs=4))
        xpool = ctx.enter_context(tc.tile_pool(name="xpool", bufs=2))
        upool = ctx.enter_context(tc.tile_pool(name="upool", bufs=2))
        spool = ctx.enter_context(tc.tile_pool(name="spool", bufs=2))
        ppp = ctx.enter_context(tc.tile_pool(name="ppp", bufs=3, space="PSUM"))
        pps = ctx.enter_context(tc.tile_pool(name="pps", bufs=2, space="PSUM"))
        ppc = ctx.enter_context(tc.tile_pool(name="ppc", bufs=1, space="PSUM"))
        ppu = ctx.enter_context(tc.tile_pool(name="ppu", bufs=2, space="PSUM"))

        # constants
        wpt_sb = consts.tile([128, 4, CODE], f32r, name="wpt_sb")       # [p, kchunk, o]
        nc.sync.dma_start(wpt_sb[:], wpt_in.rearrange("(kk p) m -> p kk m", p=128))
        bp_sb = consts.tile([128, 2], f32, name="bp_sb")                # [p, half]
        nc.sync.dma_start(bp_sb[:], bp_in[:])
        qat_sb = consts.tile([128, BPC, 2, MEM], f32r, name="qat_sb")   # [p, b, kchunk, m]
        nc.sync.dma_start(qat_sb[:], qat_in.rearrange("b (kk p) m -> p b kk m", p=128))
        qa_sb = consts.tile([MEM, BPC, CODE], f32r, name="qa_sb")       # [m, b, c]
        nc.sync.dma_start(qa_sb[:], qa_in.rearrange("b m c -> m b c"))
        ones20_sb = consts.tile([MEM, 1], f32r, name="ones20_sb")
        nc.sync.dma_start(ones20_sb[:], ones20_in[:])

        x_tiles = {}
        u_tiles = {}

        def proj_chunk(b, j):
            feats_b = feats_in[b].rearrange("(kk p) n -> p kk n", p=128)
            x_sb = x_tiles[b]
            ft = fpool.tile([128, 4, NT], f32r, tag="ft")
            nc.sync.dma_start(ft[:], feats_b[:, :, j * NT:(j + 1) * NT])
            for h in range(2):
                ps = ppp.tile([128, NT], f32, tag="proj_ps")
                for kk in range(4):
                    nc.tensor.matmul(
                        ps[:], wpt_sb[:, kk, h * 128:(h + 1) * 128], ft[:, kk, :],
                        start=(kk == 0), stop=(kk == 3))
                # psum -> sbuf with per-channel bias; fp32r out rounds for PE
                if h == 0:
                    nc.scalar.activation(
                        x_sb[:, h, j * NT:(j + 1) * NT], ps[:],
                        mybir.ActivationFunctionType.Identity,
                        bias=bp_sb[:, h:h + 1])
                else:
                    nc.vector.tensor_scalar_add(
                        x_sb[:, h, j * NT:(j + 1) * NT], ps[:], bp_sb[:, h:h + 1])

        def attn_chunk(b, j):
            x_sb, u_sb = x_tiles[b], u_tiles[b]
            js = slice(j * NT, (j + 1) * NT)
            lg = pps.tile([MEM, NT], f32, tag="logit_ps")
            for kk in range(2):
                nc.tensor.matmul(lg[:], qat_sb[:, b, kk, :], x_sb[:, kk, js],
                                 start=(kk == 0), stop=(kk == 1))
            pexp = spool.tile([MEM, NT], f32r, tag="pexp")
            nc.scalar.activation(pexp[:], lg[:], mybir.ActivationFunctionType.Exp)
            cs = ppc.tile([1, NT], f32, tag="colsum_ps")
            nc.tensor.matmul(cs[:], ones20_sb[:], pexp[:], start=True, stop=True)
            # 1/colsum at ~18 correct bits (more than fp32r's mantissa), then
            # broadcast across the 20 memory-slot partitions on the
            # otherwise-idle GpSimd engine.
            rc = spool.tile([1, NT], f32, tag="recip")
            nc.vector.reciprocal_approx_fast(out=rc[:], in_=cs[:])
            rcb = spool.tile([MEM, NT], f32, tag="rcb")
            nc.gpsimd.partition_broadcast(rcb[:], rc[:], channels=MEM)
            pn = spool.tile([MEM, NT], f32r, tag="pn")
            nc.vector.tensor_mul(pn[:], pexp[:], rcb[:])
            for h in range(2):
                us = ppu.tile([128, NT], f32, tag="u_ps")
                nc.tensor.matmul(us[:], qa_sb[:, b, h * 128:(h + 1) * 128], pn[:],
                                 start=True, stop=True)
                if h == 0:
                    nc.scalar.copy(u_sb[:, h, js], us[:])
                else:
                    nc.vector.tensor_copy(u_sb[:, h, js], us[:])

        def flush_sample(b):
            # channels [0:256] = new_feat, [256:512] = x
            x_sb, u_sb = x_tiles.pop(b), u_tiles.pop(b)
            for h in range(2):
                nc.gpsimd.dma_start(out_ext[b, 256 + h * 128:256 + (h + 1) * 128, :],
                                    x_sb[:, h, :].bitcast(f32))
                nc.gpsimd.dma_start(out_ext[b, h * 128:(h + 1) * 128, :], u_sb[:, h, :])

        # Software pipeline: projection of sample b interleaves with the
        # attention of sample b-1 so the PE always has independent matmuls.
        for b in range(BPC + 1):
            if b < BPC:
                x_tiles[b] = xpool.tile([128, 2, HW], f32r, tag="x_sb", name=f"x_sb{b}")
                u_tiles[b] = upool.tile([128, 2, HW], f32, tag="u_sb", name=f"u_sb{b}")
            for j in range(NCH):
                if b < BPC:
                    proj_chunk(b, j)
                if b >= 1:
                    attn_chunk(b - 1, j)
            if b >= 1:
                flush_sample(b - 1)

    nc.compile()
    return nc


def kernel(feats, preds, labels, flag, queue, Wp, bp):
    from concourse.bass_utils import run_bass_kernel_spmd
    global LAST_RESULTS

    feats = np.ascontiguousarray(np.asarray(feats, dtype=np.float32))
    preds = np.ascontiguousarray(np.asarray(preds, dtype=np.float32))
    labels = np.asarray(labels).astype(np.int64)
    queue = np.ascontiguousarray(np.asarray(queue, dtype=np.float32))
    Wp = np.ascontiguousarray(np.asarray(Wp, dtype=np.float32))
    bp = np.ascontiguousarray(np.asarray(bp, dtype=np.float32))
    try:
        flag_v = int(np.asarray(flag))
    except TypeError:
        flag_v = int(flag)

    qfin = _host_queue_update(feats, preds, labels, flag_v, queue, Wp, bp)
    qA = np.ascontiguousarray(qfin[labels])                      # [B, 20, 256]
    qAT = np.ascontiguousarray(qA.transpose(0, 2, 1))            # [B, 256, 20]
    wpt = np.ascontiguousarray(Wp.T)                             # [512, 256]
    bpc = np.ascontiguousarray(bp.reshape(2, 128).T)
    ones20 = np.ones((MEM, 1), dtype=np.float32)

    if "prog" not in _PROGRAM_CACHE:
        _PROGRAM_CACHE["prog"] = _build_program()
    nc = _PROGRAM_CACHE["prog"]

    f4 = feats.reshape(B, IN_C, HW)
    in_maps = []
    for k in range(NCORES):
        s = slice(k * BPC, (k + 1) * BPC)
        in_maps.append({
            "feats": np.ascontiguousarray(f4[s]),
            "wpt": wpt,
            "bpc": bpc,
            "qat": np.ascontiguousarray(qAT[s]),
            "qa": np.ascontiguousarray(qA[s]),
            "ones20": ones20,
        })

    trace = bool(int(os.environ.get("KERNEL_TRACE", "0")))
    res = run_bass_kernel_spmd(nc, in_maps, core_ids=list(range(NCORES)),
                               trace=trace)
    LAST_RESULTS = res
    out = np.concatenate([res.results[k]["out"] for k in range(NCORES)], axis=0)
    return out.reshape(B, 2 * CODE, H, W_SP)


if __name__ == "__main__":
    rng = np.random.default_rng(0)
    d = np.load("/tmp/inputs.npz")
    out = kernel(d["feats"], d["preds"], d["labels"], d["flag"], d["queue"], d["Wp"], d["bp"])
    exp = np.load("/tmp/expected.npy")
    err = np.abs(out - exp)
    print("absmax err:", err.max(), "scale-rel:", err.max() / np.abs(exp).max())


# revision 20
# speedup vs baseline: 1.2088x; 1.1033x over previous
"""TRN2 Bass kernel for nn_ClassSemantic (scatter_memory).

Strategy
--------
Data-parallel over batch: core k owns samples 4k..4k+3 and runs
projection (1x1 conv) + memory-gather attention + concat, all in fp32r
(TF32-like, ~13 mantissa bits, 4x faster PE than fp32).

The sequential EMA queue update depends on the per-sample masked
feature means only, which are algebraically separable:
    feat_b = mean_hw((Wp@f + bp) * pred) = Wp @ mean_hw(f * pred) + bp * mean(pred)
The inner reduction (134 MFLOP, 0.4% of total work) is computed on the
host, then the exactly-sequential 32-step EMA scan (tiny: [4,20,256]
state) runs on the host in float64 and the final queue rows are shipped
to every core as constants.  The device never needs a collective.

Softmax over the 20 memory slots: logits are empirically in [-3, 3]
(queue rows are ~unit-norm, x ~ N(0,1)), so exp() without max
subtraction is safe.  Column sums / broadcasts across the 20-partition
axis are done with tiny ones-matmuls on the PE.
"""
import os
import numpy as np
from contextlib import ExitStack

B, IN_C, H, W_SP = 32, 512, 64, 64
CODE, CLASSES, MEM = 256, 4, 20
HW = H * W_SP              # 4096
NCORES = 8
BPC = B // NCORES          # 4 samples per core
DECAY, EPS = 0.9, 1e-12
NCH = 8                    # n-chunks per sample
NT = HW // NCH             # 512 spatial positions per chunk

_PROGRAM_CACHE = {}
LAST_RESULTS = None        # stash for test harness introspection


def _host_queue_update(feats, preds, labels, flag, queue, Wp, bp):
    """Final queue after the reference's sequential EMA scan (float64)."""
    if int(flag) != 1:
        return queue.astype(np.float32)
    f3 = feats.reshape(B, IN_C, HW)
    p2 = preds.reshape(B, HW)
    # g_b = mean_n feats_b[:, n] * pred_b[n]  (batched sgemv)
    g = np.matmul(f3, p2[:, :, None])[:, :, 0] / np.float32(HW)
    feat = g @ Wp.T.astype(np.float32) + bp[None, :] * p2.mean(axis=1)[:, None]
    q = queue.astype(np.float64)
    for i in range(B):
        l = int(labels[i])
        f = feat[i].astype(np.float64)
        slot = q[l]
        logit = slot @ f
        upd = logit[:, None] * f[None, :]
        nrm = np.sqrt((upd * upd).sum(axis=1, keepdims=True))
        upd = upd / np.maximum(nrm, EPS)
        q[l] = DECAY * slot + (1.0 - DECAY) * upd
    return q.astype(np.float32)


def _build_program():
    from concourse import bacc, mybir
    import concourse.tile as tile

    f32, f32r = mybir.dt.float32, mybir.dt.float32r
    nc = bacc.Bacc("TRN2", target_bir_lowering=False, debug=False)

    feats_in = nc.dram_tensor("feats", [BPC, IN_C, HW], f32r, kind="ExternalInput").ap()
    wpt_in = nc.dram_tensor("wpt", [IN_C, CODE], f32r, kind="ExternalInput").ap()
    bp_in = nc.dram_tensor("bpc", [128, 2], f32, kind="ExternalInput").ap()
    qat_in = nc.dram_tensor("qat", [BPC, CODE, MEM], f32r, kind="ExternalInput").ap()
    qa_in = nc.dram_tensor("qa", [BPC, MEM, CODE], f32r, kind="ExternalInput").ap()
    ones20_in = nc.dram_tensor("ones20", [MEM, 1], f32r, kind="ExternalInput").ap()
    out_ext = nc.dram_tensor("out", [BPC, 2 * CODE, HW], f32, kind="ExternalOutput").ap()

    with tile.TileContext(nc) as tc, ExitStack() as ctx:
        consts = ctx.enter_context(tc.tile_pool(name="consts", bufs=1))
        fpool = ctx.enter_context(tc.tile_pool(name="fpool", bufs=4))
        xpool = ctx.enter_context(tc.tile_pool(name="xpool", bufs=2))
        upool = ctx.enter_context(tc.tile_pool(name="upool", bufs=2))
        spool = ctx.enter_context(tc.tile_pool(name="spool", bufs=2))
        spool4 = ctx.enter_context(tc.tile_pool(name="spool4", bufs=5))
        ppp = ctx.enter_context(tc.tile_pool(name="ppp", bufs=2, space="PSUM"))
        pps = ctx.enter_context(tc.tile_pool(name="pps", bufs=2, space="PSUM"))
        ppc = ctx.enter_context(tc.tile_pool(name="ppc", bufs=2, space="PSUM"))
        ppu = ctx.enter_context(tc.tile_pool(name="ppu", bufs=2, space="PSUM"))

        # constants
        wpt_sb = consts.tile([128, 4, CODE], f32r, name="wpt_sb")       # [p, kchunk, o]
        nc.sync.dma_start(wpt_sb[:], wpt_in.rearrange("(kk p) m -> p kk m", p=128))
        bp_sb = consts.tile([128, 2], f32, name="bp_sb")                # [p, half]
        nc.sync.dma_start(bp_sb[:], bp_in[:])
        qat_sb = consts.tile([128, BPC, 2, MEM], f32r, name="qat_sb")   # [p, b, kchunk, m]
        nc.sync.dma_start(qat_sb[:], qat_in.rearrange("b (kk p) m -> p b kk m", p=128))
        qa_sb = consts.tile([MEM, BPC, CODE], f32r, name="qa_sb")       # [m, b, c]
        nc.sync.dma_start(qa_sb[:], qa_in.rearrange("b m c -> m b c"))
        ones20_sb = consts.tile([MEM, 1], f32r, name="ones20_sb")
        nc.sync.dma_start(ones20_sb[:], ones20_in[:])

        x_tiles = {}
        u_tiles = {}
        pexp_t = {}
        cs_t = {}
        pn_t = {}
        T = BPC * NCH

        def bj(c):
            return c // NCH, c % NCH

        def proj_chunk(c):
            b, j = bj(c)
            if j == 0:
                x_tiles[b] = xpool.tile([128, 2, HW], f32r, tag="x_sb", name=f"x_sb{b}")
                u_tiles[b] = upool.tile([128, 2, HW], f32, tag="u_sb", name=f"u_sb{b}")
            feats_b = feats_in[b].rearrange("(kk p) n -> p kk n", p=128)
            x_sb = x_tiles[b]
            ft = fpool.tile([128, 4, NT], f32r, tag="ft", name=f"ft{c}")
            nc.sync.dma_start(ft[:], feats_b[:, :, j * NT:(j + 1) * NT])
            for h in range(2):
                ps = ppp.tile([128, NT], f32, tag="proj_ps", name=f"pps{c}_{h}")
                for kk in range(4):
                    nc.tensor.matmul(
                        ps[:], wpt_sb[:, kk, h * 128:(h + 1) * 128], ft[:, kk, :],
                        start=(kk == 0), stop=(kk == 3))
                # psum -> sbuf with per-channel bias; fp32r out rounds for PE
                if h == 0:
                    nc.scalar.activation(
                        x_sb[:, h, j * NT:(j + 1) * NT], ps[:],
                        mybir.ActivationFunctionType.Identity,
                        bias=bp_sb[:, h:h + 1])
                else:
                    nc.vector.tensor_scalar_add(
                        x_sb[:, h, j * NT:(j + 1) * NT], ps[:], bp_sb[:, h:h + 1])
            if j == NCH - 1:
                # x half of the output can stream out as soon as it's complete
                for h in range(2):
                    nc.gpsimd.dma_start(
                        out_ext[b, 256 + h * 128:256 + (h + 1) * 128, :],
                        x_sb[:, h, :].bitcast(f32))

        def logit_stage(c):
            b, j = bj(c)
            x_sb = x_tiles[b]
            js = slice(j * NT, (j + 1) * NT)
            lg = pps.tile([MEM, NT], f32, tag="logit_ps", name=f"lg{c}")
            for kk in range(2):
                nc.tensor.matmul(lg[:], qat_sb[:, b, kk, :], x_sb[:, kk, js],
                                 start=(kk == 0), stop=(kk == 1))
            pexp = spool4.tile([MEM, NT], f32r, tag="pexp", name=f"pexp{c}")
            nc.scalar.activation(pexp[:], lg[:], mybir.ActivationFunctionType.Exp)
            pexp_t[c] = pexp

        def sum_stage(c):
            cs = ppc.tile([1, NT], f32, tag="colsum_ps", name=f"cs{c}")
            nc.tensor.matmul(cs[:], ones20_sb[:], pexp_t[c][:], start=True, stop=True)
            cs_t[c] = cs

        def recip_stage(c):
            # 1/colsum at ~18 correct bits (more than fp32r's mantissa), then
            # broadcast across the 20 memory-slot partitions on the
            # otherwise-idle GpSimd engine.
            rc = spool.tile([1, NT], f32, tag="recip", name=f"rc{c}")
            nc.vector.reciprocal_approx_fast(out=rc[:], in_=cs_t.pop(c)[:])
            rcb = spool.tile([MEM, NT], f32, tag="rcb", name=f"rcb{c}")
            nc.gpsimd.partition_broadcast(rcb[:], rc[:], channels=MEM)
            pn_t[c] = (rcb,)

        def u_stage(c):
            b, j = bj(c)
            u_sb = u_tiles[b]
            js = slice(j * NT, (j + 1) * NT)
            (rcb,) = pn_t.pop(c)
            pn = spool.tile([MEM, NT], f32r, tag="pn", name=f"pn{c}")
            nc.vector.tensor_mul(pn[:], pexp_t.pop(c)[:], rcb[:])
            for h in range(2):
                us = ppu.tile([128, NT], f32, tag="u_ps", name=f"us{c}_{h}")
                nc.tensor.matmul(us[:], qa_sb[:, b, h * 128:(h + 1) * 128], pn[:],
                                 start=True, stop=True)
                if h == 0:
                    nc.scalar.copy(u_sb[:, h, js], us[:])
                else:
                    nc.vector.tensor_copy(u_sb[:, h, js], us[:])

        def u_flush(c):
            # flush chunks c-1, c of new_feat
            b, j = bj(c)
            u_sb = u_tiles[b]
            js2 = slice((j - 1) * NT, (j + 1) * NT)
            for h in range(2):
                nc.gpsimd.dma_start(out_ext[b, h * 128:(h + 1) * 128, (j - 1) * NT:(j + 1) * NT],
                                    u_sb[:, h, js2])

        # Chunk-level software pipeline: stage s of chunk c is emitted at
        # iteration c+s, so every cross-engine hop has a full iteration of
        # slack and the PE stream never waits on the softmax chain.
        for t in range(T + 6):
            if t < T:
                proj_chunk(t)
            if 0 <= t - 1 < T:
                logit_stage(t - 1)
            if 0 <= t - 2 < T:
                sum_stage(t - 2)
            if 0 <= t - 3 < T:
                recip_stage(t - 3)
            if 0 <= t - 4 < T:
                u_stage(t - 4)
            if 0 <= t - 5 < T and (t - 5) % 2 == 1:
                u_flush(t - 5)

    nc.compile()
    return nc


def kernel(feats, preds, labels, flag, queue, Wp, bp):
    from concourse.bass_utils import run_bass_kernel_spmd
    global LAST_RESULTS

    feats = np.ascontiguousarray(np.asarray(feats, dtype=np.float32))
    preds = np.ascontiguousarray(np.asarray(preds, dtype=np.float32))
    labels = np.asarray(labels).astype(np.int64)
    queue = np.ascontiguousarray(np.asarray(queue, dtype=np.float32))
    Wp = np.ascontiguousarray(np.asarray(Wp, dtype=np.float32))
    bp = np.ascontiguousarray(np.asarray(bp, dtype=np.float32))
    try:
        flag_v = int(np.asarray(flag))
    except TypeError:
        flag_v = int(flag)

    qfin = _host_queue_update(feats, preds, labels, flag_v, queue, Wp, bp)
    qA = np.ascontiguousarray(qfin[labels])                      # [B, 20, 256]
    qAT = np.ascontiguousarray(qA.transpose(0, 2, 1))            # [B, 256, 20]
    wpt = np.ascontiguousarray(Wp.T)                             # [512, 256]
    bpc = np.ascontiguousarray(bp.reshape(2, 128).T)
    ones20 = np.ones((MEM, 1), dtype=np.float32)

    if "prog" not in _PROGRAM_CACHE:
        _PROGRAM_CACHE["prog"] = _build_program()
    nc = _PROGRAM_CACHE["prog"]

    f4 = feats.reshape(B, IN_C, HW)
    in_maps = []
    for k in range(NCORES):
        s = slice(k * BPC, (k + 1) * BPC)
        in_maps.append({
            "feats": np.ascontiguousarray(f4[s]),
            "wpt": wpt,
            "bpc": bpc,
            "qat": np.ascontiguousarray(qAT[s]),
            "qa": np.ascontiguousarray(qA[s]),
            "ones20": ones20,
        })

    trace = bool(int(os.environ.get("KERNEL_TRACE", "0")))
    res = run_bass_kernel_spmd(nc, in_maps, core_ids=list(range(NCORES)),
                               trace=trace)
    LAST_RESULTS = res
    out = np.concatenate([res.results[k]["out"] for k in range(NCORES)], axis=0)
    return out.reshape(B, 2 * CODE, H, W_SP)


if __name__ == "__main__":
    rng = np.random.default_rng(0)
    d = np.load("/tmp/inputs.npz")
    out = kernel(d["feats"], d["preds"], d["labels"], d["flag"], d["queue"], d["Wp"], d["bp"])
    exp = np.load("/tmp/expected.npy")
    err = np.abs(out - exp)
    print("absmax err:", err.max(), "scale-rel:", err.max() / np.abs(exp).max())
